# revision 14
# baseline (speedup 1.0000x reference)
"""LIF neuron scan kernel for Trainium2 (8 NeuronCores, SPMD).

Reference semantics (per element, scan over T):
    H[t] = V[t-1] - (V[t-1] - 0.5)/2 + x[t]
    S[t] = (H[t] >= 1.0)
    V[t] = S[t] ? 0.5 : H[t]

Kernel formulation with g[t] = H[t] - 0.5 (bit-identical in fp32):
    g[0]   = x[0]
    S[t]   = (g[t] >= 0.5)
    g[t+1] = F(g[t]) + x[t+1],   F(g) = 0.5 * g * [g < 0.5]

The baseline computes F plus the add as TWO serial DVE ops per step
(DVE 95% busy, 153us).  Here the reset map F is evaluated on the
*Activation engine* via a repurposed PWP activation table: the compiler
reads activation tables from --act-root-json (override via the
documented BASS_ACT_ROOT_JSON_PATH env var), and the Gelu slot of the
"gelu_and_others" table set is rewritten so that every bucket evaluates
our F exactly:

    buckets with anchor a < 0.5 (all binades below 0.5, both signs):
        [c0, c1, c2, c3, a] = [0.5*a, 0.5, 0, 0, a]
        -> eval c0 + c1*(x - a) = 0.5*x, EXACT in fp32: x and a share a
           binade so (x - a) is exact (Sterbenz), 0.5 scaling is exact,
           and 0.5(x-a) + 0.5a = 0.5x is representable.
    buckets with anchor >= 0.5 and the large-positive control: all-zero
        -> 0, exact.
    small-signal controls / large-negative control: [0, 0.5, 0, 0, 0]
        -> 0.5*x, exact.

So ACT computes the whole nonlinearity in one pass and DVE only does
    g' = (F mult 1.0) add x'      (one scalar_tensor_tensor per chunk)
halving the serial-chain cost on DVE.  Columns are processed in two
512-wide chunks so the ACT->DVE->ACT dependency cycle (one chunk) fits
inside the step period (pipelining across chunks).

Spikes: S = [g >= 0.5] <=> F(g) == 0 (up to the measure-zero g == +-0
case).  They are emitted as int8 from both engines, split to balance
load:  DVE: (F[:, :SD] is_equal 0) -> {1,0};  ACT: Sign(F[:, SD:]) ->
{-1,0,+1} where 0 means spike.  The host decodes the two column ranges
accordingly.  Data-parallel over (B*N) across 8 cores; no cross-device
communication.
"""

import json
import os
import shutil
import sys
import tempfile

import numpy as np

if "/opt/trn_rl_repo" not in sys.path:
    sys.path.insert(0, "/opt/trn_rl_repo")

import bass_rust
import concourse.bass as bass
import concourse.mybir as mybir
import concourse.tile as tile
from concourse.bass_utils import run_bass_kernel_spmd

T, B, N = 64, 32, 32768
NCORES = 8
BN = B * N
PER = BN // NCORES  # 131072 elements per core per timestep
P = 128
F = PER // P  # 1024
FC = 512  # chunk width (two chunks pipeline the ACT<->DVE chain)
SD = 776  # spike columns [0, SD) via DVE is_equal; [SD, F) via ACT Sign

_CACHE = {}


def _install_lif_act_table() -> None:
    """Write a patched copy of the PWP activation tables in which the
    Gelu slot of gelu_and_others computes F(x) = 0.5*x*[x < 0.5] exactly
    (see module docstring), and point the compiler at it via the
    documented BASS_ACT_ROOT_JSON_PATH override."""
    if os.environ.get("LIF_NO_TABLE_PATCH"):  # debug escape hatch
        return
    if os.environ.get("BASS_ACT_ROOT_JSON_PATH", "").endswith(
        "lif_act_tables/act_info.json"
    ):
        return
    from neuronxcc.driver.Job import Job
    from neuronxcc.driver.jobs.support.FindActInfo import findActInfoFile

    src = os.path.dirname(findActInfoFile(Job.getPackageDir(), "core_v4"))
    dst = os.path.join(tempfile.gettempdir(), "lif_act_tables")
    if not os.path.exists(os.path.join(dst, "act_info.json")):
        tmp = tempfile.mkdtemp(dir=tempfile.gettempdir())
        shutil.copytree(src, tmp, dirs_exist_ok=True)
        bkt_path = os.path.join(tmp, "gelu_and_others_bkt.bin")
        a = np.fromfile(bkt_path, dtype=np.float32).reshape(-1, 8).copy()
        meta = json.load(open(os.path.join(tmp, "gelu_and_others.json")))
        n_gelu = meta["func_to_bkt_start_idx"]["derivative_gelu"]  # 508
        half = np.float32(0.5)
        for i in range(n_gelu - 4):  # regular per-binade buckets
            anchor = np.float32(a[i, 4])
            if anchor < half:
                a[i] = [half * anchor, half, 0.0, 0.0, anchor, 0.0, 0.0, 0.0]
            else:
                a[i] = [0.0, 0.0, 0.0, 0.0, anchor, 0.0, 0.0, 0.0]
        lin0 = [0.0, half, 0.0, 0.0, 0.0, 0.0, 0.0, 0.0]  # 0.5*x, anchor 0
        a[n_gelu - 4] = lin0  # small-positive control bucket
        a[n_gelu - 3] = lin0  # small-negative control bucket
        a[n_gelu - 2] = 0.0  # large-positive control bucket -> 0
        a[n_gelu - 1] = lin0  # large-negative control bucket -> 0.5*x
        a.astype(np.float32).tofile(bkt_path)
        shutil.rmtree(dst, ignore_errors=True)
        try:
            os.replace(tmp, dst)
        except OSError:  # lost a race with a concurrent builder
            shutil.rmtree(tmp, ignore_errors=True)
    os.environ["BASS_ACT_ROOT_JSON_PATH"] = os.path.join(dst, "act_info.json")


def _strip_same_engine_waits(nc: bass.Bass) -> None:
    """Remove waits on an engine's OWN tile semaphore from that engine's
    instructions.  The compute engines execute in order and the hardware
    drains each op's pipeline before the next issues, so same-engine
    RAW/WAR ordering through SBUF is already guaranteed; the semaphore
    wait only adds the @complete-ack latency (~100ns) per op.  Cross-
    engine waits (other engines' sems, DMAHW) are untouched."""
    own = {
        mybir.EngineType.DVE: "DVE_",
        mybir.EngineType.Activation: "Activation_",
    }
    for f in nc.m.functions:
        for blk in f.blocks:
            for inst in blk.instructions:
                si = inst.sync_info
                if si is None or not si.on_wait:
                    continue
                pref = own.get(inst.engine)
                if pref is None:
                    continue
                keep = [
                    w
                    for w in si.on_wait
                    if not (getattr(w, "ant_name", "") or "").startswith(pref)
                ]
                if len(keep) != len(si.on_wait):
                    si.on_wait = keep


def _hoist_excess_waits(nc: bass.Bass, limit: int = 1) -> None:
    """For instructions carrying more waits than the codegen allows, move
    excess waits onto the nearest PRECEDING same-engine instruction that
    has spare wait slots.  Waiting earlier in the same in-order stream is
    strictly more conservative, so semantics are preserved, and it avoids
    spending a NoOp slot on the engine queue."""
    for f in nc.m.functions:
        for blk in f.blocks:
            last_by_engine: dict = {}
            for inst in blk.instructions:
                si = inst.sync_info
                if si is not None and len(si.on_wait) > limit:
                    prev = last_by_engine.get(inst.engine)
                    if prev is not None:
                        psi = prev.sync_info
                        if psi is None:
                            psi = bass_rust.SyncInfo(on_wait=[], on_update=[])
                            prev.sync_info = psi
                        waits = list(si.on_wait)
                        spare = limit - len(psi.on_wait)
                        if spare > 0:
                            moved, rest = waits[:spare], waits[spare:]
                            psi.on_wait = list(psi.on_wait) + moved
                            si.on_wait = rest
                if inst.opcode not in ("UnconditionalBranch", "CompareBranch"):
                    last_by_engine[inst.engine] = inst
            last_by_engine.clear()


def _neuter_drains(nc: bass.Bass, engines=(mybir.EngineType.DVE,)) -> None:
    """Replace Drain instructions on the given engines with NoOps (keeping
    their barrier sync_info).  The terminal tile-context Drain on DVE
    measures ~10us on hardware; nothing downstream needs it — the last
    consumer of the data synchronizes via the tile semaphores, and output
    integrity is guaranteed by the runtime's own DMA-ring drain at NEFF
    completion."""
    n = 0
    for f in nc.m.functions:
        for blk in f.blocks:
            out = []
            for inst in blk.instructions:
                if inst.opcode == "Drain" and inst.engine in engines:
                    nop = bass_rust.InstNoOp(name=f"I-nodrain-{n}")
                    n += 1
                    nop.engine = inst.engine
                    nop.sync_info = inst.sync_info
                    out.append(nop)
                else:
                    out.append(inst)
            blk.instructions = out


def _split_excess_waits(nc: bass.Bass, limit: int = 1) -> None:
    """This walrus codegen rejects any instruction carrying more than one
    sync-wait command.  Move the excess waits onto same-engine NoOps
    inserted immediately before the offending instruction — semantically
    identical, the engine just performs the waits one slot earlier in its
    own stream (one wait per NoOp)."""
    n = 0
    for f in nc.m.functions:
        for blk in f.blocks:
            insts = blk.instructions
            out = []
            for inst in insts:
                si = inst.sync_info
                if si is not None and len(si.on_wait) > limit:
                    waits = list(si.on_wait)
                    excess, keep = waits[:-limit], waits[-limit:]
                    for w in excess:
                        nop = bass_rust.InstNoOp(name=f"I-waitnop-{n}")
                        n += 1
                        nop.engine = inst.engine
                        nop.sync_info = bass_rust.SyncInfo(
                            on_wait=[w], on_update=[]
                        )
                        out.append(nop)
                    si.on_wait = keep
                out.append(inst)
            blk.instructions = out


def build_nc() -> bass.Bass:
    _install_lif_act_table()
    nc = bass.Bass()
    f32 = mybir.dt.float32
    i8 = mybir.dt.int8
    x = nc.dram_tensor("x", [T, P, F], f32, kind="ExternalInput")
    s = nc.dram_tensor("s", [T, P, F], i8, kind="ExternalOutput")

    alu = mybir.AluOpType
    Gelu = mybir.ActivationFunctionType.Gelu  # repurposed: F(g)=0.5g[g<0.5]
    Sign = mybir.ActivationFunctionType.Sign
    with tile.TileContext(nc) as tc:
        with (
            tc.tile_pool(name="xin", bufs=12) as xpool,
            tc.tile_pool(name="fg", bufs=3) as fpool,
            tc.tile_pool(name="g", bufs=3) as gpool,
            tc.tile_pool(name="sout", bufs=8) as spool,
            tc.tile_pool(name="consts", bufs=1) as cpool,
        ):
            # Register constants for activation biases so the framework
            # doesn't emit raw consts + all-engine barriers.
            zero = cpool.tile([P, 1], f32)
            nc.vector.memset(zero[:], 0.0)
            nc.const_aps.aps[(f32, 0.0)] = zero[:]

            xn = xpool.tile([P, F], f32)
            nc.sync.dma_start(xn[:], x[0])
            # g[0] = x[0]: chunk views straight into the x tile.
            ga, gb = xn[:, 0:FC], xn[:, FC:F]
            for t in range(T):
                # ACT: Fg = F(g) for both chunks, into one [P, F] tile.
                fg = fpool.tile([P, F], f32)
                nc.scalar.activation(fg[:, 0:FC], ga, Gelu, bias=0.0)
                nc.scalar.activation(fg[:, FC:F], gb, Gelu, bias=0.0)
                # Spikes from Fg: F(g) == 0  <=>  g >= 0.5  (g == +-0 is
                # measure-zero).  DVE covers [0, SD), ACT covers [SD, F).
                # DVE order is sttA, sigma, sttB: sttA needs only the
                # first F-chunk, sigma needs both, sttB's input is ready
                # by the time sttA+sigma have run - no serialization.
                st = spool.tile([P, F], i8)
                gn = None
                if t + 1 < T:
                    gn = gpool.tile([P, F], f32, tag="gn")
                    xn = xpool.tile([P, F], f32)
                    nc.sync.dma_start(xn[:], x[t + 1])
                    nc.vector.scalar_tensor_tensor(
                        gn[:, 0:FC], fg[:, 0:FC], 1.0, xn[:, 0:FC],
                        alu.mult, alu.add,
                    )
                nc.vector.tensor_scalar(
                    st[:, 0:SD], fg[:, 0:SD], 0.0, None, alu.is_equal
                )
                nc.scalar.activation(st[:, SD:F], fg[:, SD:F], Sign, bias=0.0)
                if gn is not None:
                    nc.vector.scalar_tensor_tensor(
                        gn[:, FC:F], fg[:, FC:F], 1.0, xn[:, FC:F],
                        alu.mult, alu.add,
                    )
                    ga, gb = gn[:, 0:FC], gn[:, FC:F]
                # Stores ride the Pool engine's SWDGE queue so the input
                # loads have the SP HWDGE ring to themselves (the single
                # shared ring capped effective DMA at ~276 GB/s).
                nc.gpsimd.dma_start(s[t], st[:])
    _strip_same_engine_waits(nc)
    _neuter_drains(
        nc,
        engines=(
            mybir.EngineType.DVE,
            mybir.EngineType.Activation,
            mybir.EngineType.Pool,
            mybir.EngineType.PE,
        ),
    )
    # NOTE: _hoist_excess_waits is NOT used here: moving a wait onto the
    # preceding same-engine instruction deadlocks this kernel's ACT<->DVE
    # ping-pong (e.g. stt(t-1) would wait on the fg(t) semaphore whose
    # producer needs stt(t-1)'s output).  _split_excess_waits inserts the
    # wait as a NoOp *between* the two instructions, which cannot deadlock.
    _split_excess_waits(nc)
    return nc


def _get_nc() -> bass.Bass:
    if "nc" not in _CACHE:
        _CACHE["nc"] = build_nc()
    return _CACHE["nc"]


def kernel(x: np.ndarray, **run_kwargs):
    x = np.asarray(x)
    assert x.shape == (T, B, N), x.shape
    assert x.dtype == np.float32, x.dtype
    xf = x.reshape(T, BN)
    in_maps = [
        {"x": np.ascontiguousarray(xf[:, k * PER : (k + 1) * PER]).reshape(T, P, F)}
        for k in range(NCORES)
    ]
    res = run_bass_kernel_spmd(_get_nc(), in_maps, list(range(NCORES)), **run_kwargs)
    out = np.empty((T, BN), dtype=np.float32)
    for k in range(NCORES):
        sk = res.results[k]["s"].reshape(T, P, F)
        dec = np.empty((T, P, F), dtype=np.float32)
        # [0, SD): DVE is_equal -> 1 means spike.  [SD, F): ACT Sign of
        # F(g) -> 0 means spike (F(g) == 0), +-1 means no spike.
        dec[:, :, 0:SD] = sk[:, :, 0:SD] > 0
        dec[:, :, SD:F] = sk[:, :, SD:F] == 0
        out[:, k * PER : (k + 1) * PER] = dec.reshape(T, PER)
    out = out.reshape(T, B, N)
    if run_kwargs:
        return out, res
    return out


# revision 15
# speedup vs baseline: 1.1506x; 1.1506x over previous
"""LIF neuron scan kernel for Trainium2 (8 NeuronCores, SPMD).

Reference semantics (per element, scan over T):
    H[t] = V[t-1] - (V[t-1] - 0.5)/2 + x[t]
    S[t] = (H[t] >= 1.0)
    V[t] = S[t] ? 0.5 : H[t]

Kernel formulation with g[t] = H[t] - 0.5 (bit-identical in fp32):
    g[0]   = x[0]
    S[t]   = (g[t] >= 0.5)
    g[t+1] = F(g[t]) + x[t+1],   F(g) = 0.5 * g * [g < 0.5]

The baseline computes F plus the add as TWO serial DVE ops per step
(DVE 95% busy, 153us).  Here the reset map F is evaluated on the
*Activation engine* via a repurposed PWP activation table: the compiler
reads activation tables from --act-root-json (override via the
documented BASS_ACT_ROOT_JSON_PATH env var), and the Gelu slot of the
"gelu_and_others" table set is rewritten so that every bucket evaluates
our F exactly:

    buckets with anchor a < 0.5 (all binades below 0.5, both signs):
        [c0, c1, c2, c3, a] = [0.5*a, 0.5, 0, 0, a]
        -> eval c0 + c1*(x - a) = 0.5*x, EXACT in fp32: x and a share a
           binade so (x - a) is exact (Sterbenz), 0.5 scaling is exact,
           and 0.5(x-a) + 0.5a = 0.5x is representable.
    buckets with anchor >= 0.5 and the large-positive control: all-zero
        -> 0, exact.
    small-signal controls / large-negative control: [0, 0.5, 0, 0, 0]
        -> 0.5*x, exact.

So ACT computes the whole nonlinearity in one pass and DVE only does
    g' = (F mult 1.0) add x'      (one scalar_tensor_tensor per chunk)
halving the serial-chain cost on DVE.  Columns are processed in two
512-wide chunks so the ACT->DVE->ACT dependency cycle (one chunk) fits
inside the step period (pipelining across chunks).

Spikes: S = [g >= 0.5] <=> F(g) == 0 (up to the measure-zero g == +-0
case).  They are emitted as int8 from both engines, split to balance
load:  DVE: (F[:, :SD] is_equal 0) -> {1,0};  ACT: Sign(F[:, SD:]) ->
{-1,0,+1} where 0 means spike.  The host decodes the two column ranges
accordingly.  Data-parallel over (B*N) across 8 cores; no cross-device
communication.
"""

import json
import os
import shutil
import sys
import tempfile

import numpy as np

if "/opt/trn_rl_repo" not in sys.path:
    sys.path.insert(0, "/opt/trn_rl_repo")

import bass_rust
import concourse.bass as bass
import concourse.mybir as mybir
import concourse.tile as tile
from concourse.bass_utils import run_bass_kernel_spmd

T, B, N = 64, 32, 32768
NCORES = 8
BN = B * N
PER = BN // NCORES  # 131072 elements per core per timestep
P = 128
F = PER // P  # 1024
FC = 512  # chunk width (two chunks pipeline the ACT<->DVE chain)
SD = 776  # spike columns [0, SD) via DVE is_equal; [SD, F) via ACT Sign

_CACHE = {}


def _install_lif_act_table() -> None:
    """Write a patched copy of the PWP activation tables in which the
    Gelu slot of gelu_and_others computes F(x) = 0.5*x*[x < 0.5] exactly
    (see module docstring), and point the compiler at it via the
    documented BASS_ACT_ROOT_JSON_PATH override."""
    if os.environ.get("LIF_NO_TABLE_PATCH"):  # debug escape hatch
        return
    if os.environ.get("BASS_ACT_ROOT_JSON_PATH", "").endswith(
        "lif_act_tables/act_info.json"
    ):
        return
    from neuronxcc.driver.Job import Job
    from neuronxcc.driver.jobs.support.FindActInfo import findActInfoFile

    src = os.path.dirname(findActInfoFile(Job.getPackageDir(), "core_v4"))
    dst = os.path.join(tempfile.gettempdir(), "lif_act_tables")
    if not os.path.exists(os.path.join(dst, "act_info.json")):
        tmp = tempfile.mkdtemp(dir=tempfile.gettempdir())
        shutil.copytree(src, tmp, dirs_exist_ok=True)
        bkt_path = os.path.join(tmp, "gelu_and_others_bkt.bin")
        a = np.fromfile(bkt_path, dtype=np.float32).reshape(-1, 8).copy()
        meta = json.load(open(os.path.join(tmp, "gelu_and_others.json")))
        n_gelu = meta["func_to_bkt_start_idx"]["derivative_gelu"]  # 508
        half = np.float32(0.5)
        for i in range(n_gelu - 4):  # regular per-binade buckets
            anchor = np.float32(a[i, 4])
            if anchor < half:
                a[i] = [half * anchor, half, 0.0, 0.0, anchor, 0.0, 0.0, 0.0]
            else:
                a[i] = [0.0, 0.0, 0.0, 0.0, anchor, 0.0, 0.0, 0.0]
        lin0 = [0.0, half, 0.0, 0.0, 0.0, 0.0, 0.0, 0.0]  # 0.5*x, anchor 0
        a[n_gelu - 4] = lin0  # small-positive control bucket
        a[n_gelu - 3] = lin0  # small-negative control bucket
        a[n_gelu - 2] = 0.0  # large-positive control bucket -> 0
        a[n_gelu - 1] = lin0  # large-negative control bucket -> 0.5*x
        a.astype(np.float32).tofile(bkt_path)
        shutil.rmtree(dst, ignore_errors=True)
        try:
            os.replace(tmp, dst)
        except OSError:  # lost a race with a concurrent builder
            shutil.rmtree(tmp, ignore_errors=True)
    os.environ["BASS_ACT_ROOT_JSON_PATH"] = os.path.join(dst, "act_info.json")


def _strip_same_engine_waits(nc: bass.Bass) -> None:
    """Remove waits on an engine's OWN tile semaphore from that engine's
    instructions.  The compute engines execute in order and the hardware
    drains each op's pipeline before the next issues, so same-engine
    RAW/WAR ordering through SBUF is already guaranteed; the semaphore
    wait only adds the @complete-ack latency (~100ns) per op.  Cross-
    engine waits (other engines' sems, DMAHW) are untouched."""
    own = {
        mybir.EngineType.DVE: "DVE_",
        mybir.EngineType.Activation: "Activation_",
    }
    for f in nc.m.functions:
        for blk in f.blocks:
            for inst in blk.instructions:
                si = inst.sync_info
                if si is None or not si.on_wait:
                    continue
                pref = own.get(inst.engine)
                if pref is None:
                    continue
                keep = [
                    w
                    for w in si.on_wait
                    if not (getattr(w, "ant_name", "") or "").startswith(pref)
                ]
                if len(keep) != len(si.on_wait):
                    si.on_wait = keep


def _hoist_excess_waits(nc: bass.Bass, limit: int = 1) -> None:
    """For instructions carrying more waits than the codegen allows, move
    excess waits onto the nearest PRECEDING same-engine instruction that
    has spare wait slots.  Waiting earlier in the same in-order stream is
    strictly more conservative, so semantics are preserved, and it avoids
    spending a NoOp slot on the engine queue."""
    for f in nc.m.functions:
        for blk in f.blocks:
            last_by_engine: dict = {}
            for inst in blk.instructions:
                si = inst.sync_info
                if si is not None and len(si.on_wait) > limit:
                    prev = last_by_engine.get(inst.engine)
                    if prev is not None:
                        psi = prev.sync_info
                        if psi is None:
                            psi = bass_rust.SyncInfo(on_wait=[], on_update=[])
                            prev.sync_info = psi
                        waits = list(si.on_wait)
                        spare = limit - len(psi.on_wait)
                        if spare > 0:
                            moved, rest = waits[:spare], waits[spare:]
                            psi.on_wait = list(psi.on_wait) + moved
                            si.on_wait = rest
                if inst.opcode not in ("UnconditionalBranch", "CompareBranch"):
                    last_by_engine[inst.engine] = inst
            last_by_engine.clear()


def _neuter_drains(nc: bass.Bass, engines=(mybir.EngineType.DVE,)) -> None:
    """Replace Drain instructions on the given engines with NoOps (keeping
    their barrier sync_info).  The terminal tile-context Drain on DVE
    measures ~10us on hardware; nothing downstream needs it — the last
    consumer of the data synchronizes via the tile semaphores, and output
    integrity is guaranteed by the runtime's own DMA-ring drain at NEFF
    completion."""
    n = 0
    for f in nc.m.functions:
        for blk in f.blocks:
            out = []
            for inst in blk.instructions:
                if inst.opcode == "Drain" and inst.engine in engines:
                    nop = bass_rust.InstNoOp(name=f"I-nodrain-{n}")
                    n += 1
                    nop.engine = inst.engine
                    nop.sync_info = inst.sync_info
                    out.append(nop)
                else:
                    out.append(inst)
            blk.instructions = out


def _split_excess_waits(nc: bass.Bass, limit: int = 1) -> None:
    """This walrus codegen rejects any instruction carrying more than one
    sync-wait command.  Move the excess waits onto same-engine NoOps
    inserted immediately before the offending instruction — semantically
    identical, the engine just performs the waits one slot earlier in its
    own stream (one wait per NoOp)."""
    n = 0
    for f in nc.m.functions:
        for blk in f.blocks:
            insts = blk.instructions
            out = []
            for inst in insts:
                si = inst.sync_info
                if si is not None and len(si.on_wait) > limit:
                    waits = list(si.on_wait)
                    excess, keep = waits[:-limit], waits[-limit:]
                    for w in excess:
                        nop = bass_rust.InstNoOp(name=f"I-waitnop-{n}")
                        n += 1
                        nop.engine = inst.engine
                        nop.sync_info = bass_rust.SyncInfo(
                            on_wait=[w], on_update=[]
                        )
                        out.append(nop)
                    si.on_wait = keep
                out.append(inst)
            blk.instructions = out


def build_nc() -> bass.Bass:
    _install_lif_act_table()
    nc = bass.Bass()
    f32 = mybir.dt.float32
    i8 = mybir.dt.int8
    x = nc.dram_tensor("x", [T, P, F], f32, kind="ExternalInput")
    s = nc.dram_tensor("s", [T, P, F], i8, kind="ExternalOutput")

    alu = mybir.AluOpType
    Gelu = mybir.ActivationFunctionType.Gelu  # repurposed: F(g)=0.5g[g<0.5]
    Sign = mybir.ActivationFunctionType.Sign
    with tile.TileContext(nc) as tc:
        with (
            tc.tile_pool(name="xin", bufs=12) as xpool,
            tc.tile_pool(name="fg", bufs=3) as fpool,
            tc.tile_pool(name="g", bufs=3) as gpool,
            tc.tile_pool(name="sout", bufs=8) as spool,
            tc.tile_pool(name="consts", bufs=1) as cpool,
        ):
            # Register constants for activation biases so the framework
            # doesn't emit raw consts + all-engine barriers.
            zero = cpool.tile([P, 1], f32)
            nc.vector.memset(zero[:], 0.0)
            nc.const_aps.aps[(f32, 0.0)] = zero[:]

            xn = xpool.tile([P, F], f32)
            nc.sync.dma_start(xn[:], x[0])
            # g[0] = x[0]: chunk views straight into the x tile.
            ga, gb = xn[:, 0:FC], xn[:, FC:F]
            for t in range(T):
                # ACT: Fg = F(g) for both chunks, into one [P, F] tile.
                fg = fpool.tile([P, F], f32)
                nc.scalar.activation(fg[:, 0:FC], ga, Gelu, bias=0.0)
                nc.scalar.activation(fg[:, FC:F], gb, Gelu, bias=0.0)
                # Spikes from Fg: F(g) == 0  <=>  g >= 0.5  (g == +-0 is
                # measure-zero).  DVE covers [0, SD), ACT covers [SD, F).
                st = spool.tile([P, F], i8)
                nc.vector.tensor_scalar(
                    st[:, 0:SD], fg[:, 0:SD], 0.0, None, alu.is_equal
                )
                nc.scalar.activation(st[:, SD:F], fg[:, SD:F], Sign, bias=0.0)
                # Stores ride the Pool engine's SWDGE queue so the input
                # loads have the SP HWDGE ring to themselves (the single
                # shared ring capped effective DMA at ~276 GB/s).
                nc.gpsimd.dma_start(s[t], st[:])
                if t + 1 < T:
                    xn = xpool.tile([P, F], f32)
                    nc.sync.dma_start(xn[:], x[t + 1])
                    # DVE: g' = Fg + x', one STT per chunk.
                    gn = gpool.tile([P, F], f32)
                    nc.vector.scalar_tensor_tensor(
                        gn[:, 0:FC], fg[:, 0:FC], 1.0, xn[:, 0:FC],
                        alu.mult, alu.add,
                    )
                    nc.vector.scalar_tensor_tensor(
                        gn[:, FC:F], fg[:, FC:F], 1.0, xn[:, FC:F],
                        alu.mult, alu.add,
                    )
                    ga, gb = gn[:, 0:FC], gn[:, FC:F]
    _strip_same_engine_waits(nc)
    _neuter_drains(nc)
    # NOTE: _hoist_excess_waits is NOT used here: moving a wait onto the
    # preceding same-engine instruction deadlocks this kernel's ACT<->DVE
    # ping-pong (e.g. stt(t-1) would wait on the fg(t) semaphore whose
    # producer needs stt(t-1)'s output).  _split_excess_waits inserts the
    # wait as a NoOp *between* the two instructions, which cannot deadlock.
    _split_excess_waits(nc)
    return nc


def _get_nc() -> bass.Bass:
    if "nc" not in _CACHE:
        _CACHE["nc"] = build_nc()
    return _CACHE["nc"]


def kernel(x: np.ndarray, **run_kwargs):
    x = np.asarray(x)
    assert x.shape == (T, B, N), x.shape
    assert x.dtype == np.float32, x.dtype
    xf = x.reshape(T, BN)
    in_maps = [
        {"x": np.ascontiguousarray(xf[:, k * PER : (k + 1) * PER]).reshape(T, P, F)}
        for k in range(NCORES)
    ]
    res = run_bass_kernel_spmd(_get_nc(), in_maps, list(range(NCORES)), **run_kwargs)
    out = np.empty((T, BN), dtype=np.float32)
    for k in range(NCORES):
        sk = res.results[k]["s"].reshape(T, P, F)
        dec = np.empty((T, P, F), dtype=np.float32)
        # [0, SD): DVE is_equal -> 1 means spike.  [SD, F): ACT Sign of
        # F(g) -> 0 means spike (F(g) == 0), +-1 means no spike.
        dec[:, :, 0:SD] = sk[:, :, 0:SD] > 0
        dec[:, :, SD:F] = sk[:, :, SD:F] == 0
        out[:, k * PER : (k + 1) * PER] = dec.reshape(T, PER)
    out = out.reshape(T, B, N)
    if run_kwargs:
        return out, res
    return out
